# revision 1
# baseline (speedup 1.0000x reference)
"""KAN layer (identity edges) Trainium2 kernel.

output[b, o] = sum_i x[b, i]  for all o  -- row-sum broadcast to (B, 1024).

Data-parallel over 8 NeuronCores: each core gets 8192 rows of x
(65536 x 1024 f32), computes row sums on the Vector engine, broadcasts
across the feature dim on-chip, and DMAs the full (8192, 1024) shard out.

Layout: partition p owns 64 consecutive DRAM rows (rearrange
"(p n) d -> p n d"), so each DMA moves R*4KB contiguous bytes per
partition.
"""

import numpy as np

import concourse.bass as bass
import concourse.tile as tile
from concourse import bacc, mybir
from concourse.bass_utils import run_bass_kernel_spmd

N_CORES = 8
BATCH = 65536
FEAT = 1024
ROWS = BATCH // N_CORES        # 8192 rows per core
P = 128                        # SBUF partitions
ROWS_PER_PART = ROWS // P      # 64 consecutive rows owned by each partition

_nc_cache = []


def _build(
    R=8,
    in_bufs=2,
    out_bufs=2,
    dma_engine="gpsimd",
    inplace=False,
    bcast_engine="vector",
):
    n_iter = ROWS_PER_PART // R
    nc = bacc.Bacc()
    x = nc.declare_dram_parameter("x", [ROWS, FEAT], mybir.dt.float32, isOutput=False)
    y = nc.declare_dram_parameter("y", [ROWS, FEAT], mybir.dt.float32, isOutput=True)
    xv = x[:, :].rearrange("(p n) d -> p n d", p=P)
    yv = y[:, :].rearrange("(p n) d -> p n d", p=P)
    dma = getattr(nc, dma_engine)
    bcast = getattr(nc, bcast_engine)

    with tile.TileContext(nc) as tc:
        with (
            tc.tile_pool(name="inp", bufs=in_bufs) as inp,
            tc.tile_pool(name="outp", bufs=out_bufs) as outp,
            tc.tile_pool(name="sums", bufs=4) as sums_pool,
        ):
            for i in range(n_iter):
                t = inp.tile([P, R, FEAT], mybir.dt.float32)
                dma.dma_start(out=t[:, :, :], in_=xv[:, i * R : (i + 1) * R, :])

                s = sums_pool.tile([P, R], mybir.dt.float32)
                nc.vector.reduce_sum(
                    out=s[:, :], in_=t[:, :, :], axis=mybir.AxisListType.X
                )

                o = t if inplace else outp.tile([P, R, FEAT], mybir.dt.float32)
                bcast.tensor_copy(
                    out=o[:, :, :], in_=s[:, :].to_broadcast([P, R, FEAT])
                )
                dma.dma_start(out=yv[:, i * R : (i + 1) * R, :], in_=o[:, :, :])
    nc.finalize()
    return nc


def _get_nc():
    if not _nc_cache:
        _nc_cache.append(_build())
    return _nc_cache[0]


def kernel(x: np.ndarray) -> np.ndarray:
    nc = _get_nc()
    x = np.ascontiguousarray(np.asarray(x), dtype=np.float32)
    shards = np.split(x, N_CORES, axis=0)
    in_maps = [{"x": s} for s in shards]
    res = run_bass_kernel_spmd(nc, in_maps, list(range(N_CORES)))
    return np.concatenate([res.results[i]["y"] for i in range(N_CORES)], axis=0)

